# revision 17
# baseline (speedup 1.0000x reference)
"""2-layer GAT (nn_GAT_31490700214331) on 8 Trainium2 NeuronCores.

Strategy (dst-sharded, SPMD, per-core-rotated node layout) — v2:
  - Nodes block-partitioned: core c owns nodes [c*6250, (c+1)*6250); every
    table on core c uses a ROTATED row order (node n at row (n - c*6250)
    mod 50000) so one SPMD program serves all cores.
  - Phase A (replicated): h0 = x @ [W0 | W0·a_src | W0·a_dst] for all
    nodes; rows [h0|as0] land in the gather tables t0lo/t0hi (512 B rows),
    dst-alphas accumulate in SBUF and are written to t0ad in one DMA.
  - Phase B: per group of 4 dst tiles, three batched dma_gathers (src rows
    lo/hi + per-edge dst-alpha); edge softmax (safe without segment-max)
    and aggregation run as 128x128 incidence matmuls; denominators ride
    as 8 fused psum columns.  Incidence builds alternate DVE/GpSimd.
  - The hidden state is ELU'd, transposed, quantized to f8e4 and
    AllGather'd in 4 column chunks that overlap phase B's tail and
    phase D's head (COLLECTIVE_CORES runs concurrently with compute).
  - Phase D: supertiles of 8 node tiles, ordered by which AllGather chunk
    they need (own-core rows first, straight from local agin); the f8
    hidden state feeds mixed-precision matmuls with W1/W1a; rows
    [h1|as1] go to t1lo/t1hi (1280 B rows), dst-alphas to t1ad.
  - Phase E: like B with 640-col rows, separate denominator chain, and a
    head-mean + batched log_softmax epilogue.
  - alpha projections fold into the weight matmuls on the host:
    h @ blockdiag(a) == x @ (W @ blockdiag(a)).

Self-contained: call kernel(**inputs) with the full-problem arrays.
"""
import numpy as np
from contextlib import ExitStack

import concourse.bacc as bacc
import concourse.bass as bass
import concourse.mybir as mybir
from concourse.tile import TileContext
from concourse.bass_utils import run_bass_kernel_spmd

F16 = mybir.dt.float16
F32 = mybir.dt.float32
F8 = mybir.dt.float8e4
I16 = mybir.dt.int16
I8 = mybir.dt.int8

N = 50000
NFEAT = 256
NHID = 128
NCLASS = 64
HEADS = 8
SLOPE = 0.2
NCORES = 8
NLOC = N // NCORES           # 6250
LT = (NLOC + 127) // 128     # 49 local dst tiles
LAST_ROWS = NLOC - (LT - 1) * 128   # 106 rows in the last tile
GT = 392                     # global node tiles (392*128 = 50176)
GROWS = GT * 128
SPLIT = 25088                # low/high gather-table split (196 tiles)
SENT = 300.0                 # dst_rel sentinel for padding slots
T0W = 256                    # t0 row: [h0(128)|as0(8)|junk] f16
T1W = 640                    # t1 row: [h1(512)|as1(8)|junk] f16
STB = 4                      # layer-0 gather supertile (dst tiles)
STE = 2                      # layer-1 gather supertile
SD = 8                       # phase-D node tiles per supertile
NCHUNK = 4                   # AllGather chunks
AGCH = {1: [49], 2: [25, 24], 4: [12, 12, 12, 13]}[NCHUNK]
AGB = list(np.cumsum([0] + AGCH))          # tile boundaries
CHB = [min(b * 128, NLOC) for b in AGB]    # col boundaries
AG_F8 = False                # f8 AllGather payload (else f16)
GBATCH = False               # multi-tile gathers (SWDGE ring risk on HW)

_cache = {}


# --------------------------------------------------------------------------
# host-side preparation
# --------------------------------------------------------------------------

def _wrap_idx(idx):
    """[n] int -> [128, n//16] int16 wrapped gather-index layout."""
    n = idx.shape[0]
    assert n % 16 == 0
    w = idx.reshape(n // 16, 16).T.astype(np.int16)
    return np.tile(w, (8, 1))


def _prep_edges(src, dst):
    cores = []
    for c in range(NCORES):
        m = (dst >= c * NLOC) & (dst < (c + 1) * NLOC)
        s = src[m].astype(np.int64)
        d = dst[m].astype(np.int64) - c * NLOC
        order = np.argsort(d, kind="stable")
        s, d = s[order], d[order]
        s_rot = (s - c * NLOC) % N
        tiles = []
        for t in range(LT):
            sel = (d >= t * 128) & (d < (t + 1) * 128)
            st, dt = s_rot[sel], d[sel] - t * 128
            lo = st < SPLIT
            tiles.append((st[lo], dt[lo], st[~lo] - SPLIT, dt[~lo]))
        cores.append(tiles)
    nl = max(len(t[0]) for tl in cores for t in tl)
    nh = max(len(t[2]) for tl in cores for t in tl)
    NL = max(1, (nl + 127) // 128)
    NH = max(1, (nh + 127) // 128)
    CH = NL + NH

    out = []
    for c in range(NCORES):
        ilb = np.zeros((LT, 128, NL * 8), np.int16)
        ihb = np.zeros((LT, 128, NH * 8), np.int16)
        aib = np.zeros((LT, 128, CH * 8), np.int16)
        drb = np.zeros((LT, 128, CH), np.int16)
        for t in range(LT):
            sl, dl, sh, dh = cores[c][t]
            il = np.zeros(NL * 128, np.int64)
            il[: len(sl)] = sl
            ih = np.zeros(NH * 128, np.int64)
            ih[: len(sh)] = sh
            ai = np.zeros(CH * 128, np.int64)
            ai[: len(dl)] = t * 128 + dl
            ai[NL * 128: NL * 128 + len(dh)] = t * 128 + dh
            ilb[t] = _wrap_idx(il)
            ihb[t] = _wrap_idx(ih)
            aib[t] = _wrap_idx(ai)
            rl = np.full(NL * 128, SENT)
            rl[: len(dl)] = dl
            rh = np.full(NH * 128, SENT)
            rh[: len(dh)] = dh
            r = np.concatenate([rl, rh]).reshape(CH, 128).T
            drb[t] = r.astype(np.float16).view(np.int16)
        epack = np.concatenate(
            [ilb.transpose(1, 0, 2).reshape(128, -1),
             ihb.transpose(1, 0, 2).reshape(128, -1),
             aib.transpose(1, 0, 2).reshape(128, -1),
             drb.transpose(1, 0, 2).reshape(128, -1)], axis=1)
        out.append(dict(epack=np.ascontiguousarray(epack)))
    return NL, NH, out


def _prep_inputs(x, edge_index, W0, a_src0, a_dst0, b0, W1, a_src1, a_dst1,
                 b1):
    src = np.asarray(edge_index[0]).astype(np.int64)
    dst = np.asarray(edge_index[1]).astype(np.int64)
    NL, NH, edata = _prep_edges(src, dst)

    def bd(a):  # [H, D] -> blockdiag [H*D, H]
        a = np.asarray(a, np.float32)
        H, D = a.shape
        m = np.zeros((H * D, H), np.float32)
        for h in range(H):
            m[h * D:(h + 1) * D, h] = a[h]
        return m

    W0 = np.asarray(W0, np.float32)
    W1 = np.asarray(W1, np.float32)
    W0a = np.concatenate([W0 @ bd(a_src0), W0 @ bd(a_dst0)], 1)  # [256, 16]
    # head-innermost feature interleave: new col d*8+h <- old col h*D+d
    perm0 = np.array([(f % 8) * 16 + f // 8 for f in range(128)])
    perm1 = np.array([(f % 8) * 64 + f // 8 for f in range(512)])
    W0cat = np.concatenate([W0[:, perm0], W0a], 1)               # [256, 144]
    W1a = np.concatenate([W1 @ bd(a_src1), W1 @ bd(a_dst1)], 1)  # [128, 16]

    x = np.asarray(x, np.float32)
    ident = np.eye(128, dtype=np.float16)
    CH = NL + NH
    colio = np.tile(np.repeat(np.arange(128, dtype=np.float16), CH)[None, :],
                    (128, 1))
    b0b = np.tile(np.asarray(b0, np.float32)[None, :], (128, 1))
    b1b = np.tile(np.asarray(b1, np.float32)[None, :], (128, 1))

    in_maps = []
    for c in range(NCORES):
        rot = np.roll(np.arange(N), -c * NLOC)
        xr = np.zeros((GROWS, NFEAT), np.float16)
        xr[:N] = x[rot].astype(np.float16)
        xtt = xr.reshape(GROWS // 128, 128, 2, 128).transpose(0, 3, 2, 1)
        m = dict(
            xT=np.ascontiguousarray(xtt),
            W0=np.ascontiguousarray(
                W0cat.astype(np.float16).reshape(2, 128, NHID + 16)),
            W1=np.ascontiguousarray(W1[perm0][:, perm1].astype(np.float16)),
            W1a=np.ascontiguousarray(W1a[perm0].astype(np.float16)),
            b0b=np.ascontiguousarray(b0b[:, perm0]), b1b=b1b,
            ident=ident, colio=colio,
            **edata[c],
        )
        in_maps.append(m)
    return NL, NH, in_maps


# --------------------------------------------------------------------------
# device program
# --------------------------------------------------------------------------

def build(NL, NH, phases="ABCDE"):
    CH = NL + NH
    HID16 = NHID + 16
    ILB = 0                       # epack col offsets (int16 cols)
    IHB = ILB + LT * NL * 8
    AIB = IHB + LT * NH * 8
    DRB = AIB + LT * CH * 8
    TOT = DRB + LT * CH

    nc = bacc.Bacc("TRN2")
    xT = nc.dram_tensor("xT", [GT, 128, 2, 128], F16, kind="ExternalInput")
    W0i = nc.dram_tensor("W0", [2, 128, HID16], F16, kind="ExternalInput")
    W1i = nc.dram_tensor("W1", [NHID, 512], F16, kind="ExternalInput")
    W1ai = nc.dram_tensor("W1a", [NHID, 16], F16, kind="ExternalInput")
    b0bi = nc.dram_tensor("b0b", [128, NHID], F32, kind="ExternalInput")
    b1bi = nc.dram_tensor("b1b", [128, NCLASS], F32, kind="ExternalInput")
    identi = nc.dram_tensor("ident", [128, 128], F16, kind="ExternalInput")
    colioi = nc.dram_tensor("colio", [128, 128 * CH], F16,
                            kind="ExternalInput")
    epacki = nc.dram_tensor("epack", [128, TOT], I16, kind="ExternalInput")
    out = nc.dram_tensor("out", [NLOC, NCLASS], F32, kind="ExternalOutput")

    with TileContext(nc) as tc, ExitStack() as stk:
        dpool = stk.enter_context(
            tc.tile_pool(name="dram", bufs=1, space="DRAM"))
        t0lo = dpool.tile([SPLIT, T0W], F16, tag="t0lo")
        t0hi = dpool.tile([GROWS - SPLIT, T0W], F16, tag="t0hi")
        t0ad = dpool.tile([LT * 128, 128], F16, tag="t0ad")
        t1lo = dpool.tile([SPLIT, T1W], F16, tag="t1lo")
        t1hi = dpool.tile([GROWS - SPLIT, T1W], F16, tag="t1hi")
        t1ad = dpool.tile([LT * 128, 128], F16, tag="t1ad")
        AGDT = I8 if AG_F8 else F16
        agin = []
        agout = []
        for k in range(NCHUNK):
            agin_k = dpool.tile([128, AGCH[k] * 128], AGDT, tag=f"agin{k}",
                                name=f"agin{k}")
            agout_k = dpool.tile([NCORES * 128, AGCH[k] * 128], AGDT,
                                 tag=f"agout{k}", addr_space="Shared",
                                 name=f"agout{k}")
            agin.append(agin_k)
            agout.append(agout_k)

        cpool = stk.enter_context(tc.tile_pool(name="const", bufs=1))
        W0s = cpool.tile([128, 2, HID16], F16)
        nc.sync.dma_start(out=W0s[:], in_=W0i.rearrange("k p n -> p k n"))
        W1s = cpool.tile([128, 512], F16)
        nc.sync.dma_start(out=W1s[:], in_=W1i[:])
        W1as = cpool.tile([128, 16], F16)
        nc.sync.dma_start(out=W1as[:], in_=W1ai[:])
        b0s = cpool.tile([128, NHID], F32)
        nc.sync.dma_start(out=b0s[:], in_=b0bi[:])
        b1s = cpool.tile([128, NCLASS], F32)
        nc.sync.dma_start(out=b1s[:], in_=b1bi[:])
        idents = cpool.tile([128, 128], F16)
        nc.sync.dma_start(out=idents[:], in_=identi[:])
        colios = cpool.tile([128, 128 * CH], F16)
        nc.sync.dma_start(out=colios[:], in_=colioi[:])
        adball = cpool.tile([128, LT * 8], F16)
        adbal2 = cpool.tile([128, LT * 8], F16)
        epS = cpool.tile([128, TOT], I16)
        nc.scalar.dma_start(out=epS[:], in_=epacki[:])

        regs = {}

        def reg(n):
            if n not in regs:
                regs[n] = nc.gpsimd.to_reg(n)
            return regs[n]

        # ---------------- phase A: layer-0 tables (replicated) ------------
        with ExitStack() as pa:
            xp = pa.enter_context(tc.tile_pool(name="pa_x", bufs=3))
            pp = pa.enter_context(
                tc.tile_pool(name="pa_ps", bufs=1, space="PSUM"))
            rp = pa.enter_context(tc.tile_pool(name="pa_row", bufs=3))
            for gg in range(GT // 8):
                xa = xp.tile([128, 8, 2, 128], F16, tag="xa")
                eng = nc.sync if gg % 2 else nc.scalar
                eng.dma_start(
                    out=xa[:],
                    in_=xT[8 * gg:8 * gg + 8].rearrange(
                        "g p k j -> p g k j"))
                row = rp.tile([128, 8, 136], F16, tag="row")
                for g2 in range(8):
                    ps = pp.tile([128, HID16], F32, tag=f"ps{g2 % 4}")
                    for k in range(2):
                        nc.tensor.matmul(ps[:], xa[:, g2, k, :],
                                         W0s[:, k, :],
                                         start=(k == 0), stop=(k == 1))
                    nc.vector.tensor_copy(row[:, g2, :], ps[:, 0:136])
                    g = 8 * gg + g2
                    if g < LT:
                        nc.vector.tensor_copy(
                            adball[:, g * 8:(g + 1) * 8], ps[:, 136:144])
                g0 = gg * 1024
                weng = nc.scalar if gg % 2 else nc.sync
                if g0 + 1024 <= SPLIT:
                    weng.dma_start(
                        out=t0lo[g0:g0 + 1024, 0:136]
                        .rearrange("(g p) w -> p g w", p=128),
                        in_=row[:])
                elif g0 >= SPLIT:
                    o = g0 - SPLIT
                    weng.dma_start(
                        out=t0hi[o:o + 1024, 0:136]
                        .rearrange("(g p) w -> p g w", p=128),
                        in_=row[:])
                else:  # straddles the split (gg == 24)
                    nlo = (SPLIT - g0) // 128
                    weng.dma_start(
                        out=t0lo[g0:SPLIT, 0:136]
                        .rearrange("(g p) w -> p g w", p=128),
                        in_=row[:, 0:nlo])
                    weng.dma_start(
                        out=t0hi[0:1024 - (SPLIT - g0), 0:136]
                        .rearrange("(g p) w -> p g w", p=128),
                        in_=row[:, nlo:8])
            nc.sync.dma_start(
                out=t0ad[:, 0:8].rearrange("(t p) w -> p t w", p=128),
                in_=adball[:].rearrange("p (t w) -> p t w", w=8))

        # ---------------- shared edge phase -------------------------------
        def edge_phase(layer, ST, tbl_lo, tbl_hi, tblad, fdim, post_fn,
                       agin_hook):
            D = fdim // HEADS
            trow = T1W if layer else T0W
            nst = (LT + ST - 1) // ST
            with ExitStack() as pb:
                gp = pb.enter_context(
                    tc.tile_pool(name=f"gg{layer}", bufs=2))
                apl = pb.enter_context(
                    tc.tile_pool(name=f"ga{layer}", bufs=2))
                incp = pb.enter_context(
                    tc.tile_pool(name=f"ic{layer}", bufs=3))
                rp2 = pb.enter_context(
                    tc.tile_pool(name=f"rh{layer}", bufs=3))
                exq = pb.enter_context(
                    tc.tile_pool(name=f"ex{layer}", bufs=3))
                pp2 = pb.enter_context(
                    tc.tile_pool(name=f"ps{layer}", bufs=3, space="PSUM"))
                op = pb.enter_context(
                    tc.tile_pool(name=f"po{layer}", bufs=3))
                for st in range(nst):
                    a, b = ST * st, min(ST * st + ST, LT)
                    nt = b - a
                    glo = gp.tile([128, ST * NL, trow], F16, tag="glo")
                    ghi = gp.tile([128, ST * NH, trow], F16, tag="ghi")
                    ga = apl.tile([128, ST * CH, 128], F16, tag="ga")
                    if GBATCH:
                        nc.gpsimd.dma_gather(
                            glo[:, 0:nt * NL, :], tbl_lo[:],
                            epS[:, ILB + a * NL * 8:ILB + b * NL * 8],
                            nt * NL * 128, reg(nt * NL * 128), trow)
                        nc.gpsimd.dma_gather(
                            ghi[:, 0:nt * NH, :], tbl_hi[:],
                            epS[:, IHB + a * NH * 8:IHB + b * NH * 8],
                            nt * NH * 128, reg(nt * NH * 128), trow)
                        nc.gpsimd.dma_gather(
                            ga[:, 0:nt * CH, :], tblad[:],
                            epS[:, AIB + a * CH * 8:AIB + b * CH * 8],
                            nt * CH * 128, reg(nt * CH * 128), 128)
                    else:
                        for t in range(a, b):
                            i = t - a
                            nc.gpsimd.dma_gather(
                                glo[:, i * NL:(i + 1) * NL, :], tbl_lo[:],
                                epS[:, ILB + t * NL * 8:
                                    ILB + (t + 1) * NL * 8],
                                NL * 128, reg(NL * 128), trow)
                            nc.gpsimd.dma_gather(
                                ghi[:, i * NH:(i + 1) * NH, :], tbl_hi[:],
                                epS[:, IHB + t * NH * 8:
                                    IHB + (t + 1) * NH * 8],
                                NH * 128, reg(NH * 128), trow)
                            nc.gpsimd.dma_gather(
                                ga[:, i * CH:i * CH + NL, :], tblad[:],
                                epS[:, AIB + t * CH * 8:
                                    AIB + t * CH * 8 + NL * 8],
                                NL * 128, reg(NL * 128), 128)
                            nc.gpsimd.dma_gather(
                                ga[:, i * CH + NL:(i + 1) * CH, :],
                                tblad[:],
                                epS[:, AIB + t * CH * 8 + NL * 8:
                                    AIB + (t + 1) * CH * 8],
                                NH * 128, reg(NH * 128), 128)
                    for t in range(a, b):
                        i = t - a
                        dr = epS[:, DRB + t * CH:DRB + (t + 1) * CH]\
                            .bitcast(F16)
                        inc = incp.tile([128, 128, CH], F16, tag="inc")
                        nc.vector.tensor_tensor(
                            out=inc[:],
                            in0=dr.unsqueeze(1)
                            .broadcast_to([128, 128, CH]),
                            in1=colios[:]
                            .rearrange("p (d c) -> p d c", c=CH),
                            op=mybir.AluOpType.is_equal)
                        EX = exq.tile([128, CH, 8], F16, tag="EX")
                        nc.vector.tensor_tensor(
                            out=EX[:, 0:NL, :],
                            in0=glo[:, i * NL:(i + 1) * NL,
                                    fdim:fdim + 8],
                            in1=ga[:, i * CH:i * CH + NL, 0:8],
                            op=mybir.AluOpType.add)
                        nc.vector.tensor_tensor(
                            out=EX[:, NL:CH, :],
                            in0=ghi[:, i * NH:(i + 1) * NH,
                                    fdim:fdim + 8],
                            in1=ga[:, i * CH + NL:(i + 1) * CH, 0:8],
                            op=mybir.AluOpType.add)
                        nc.scalar.activation(
                            EX[:], EX[:],
                            mybir.ActivationFunctionType.Prelu,
                            alpha=SLOPE)
                        nc.scalar.activation(
                            EX[:], EX[:],
                            mybir.ActivationFunctionType.Exp)
                        rw = fdim + 8 if layer == 0 else fdim
                        R = rp2.tile([128, CH, rw], F16, tag="R")
                        nc.vector.tensor_tensor(
                            out=R[:, 0:NL, 0:fdim]
                            .rearrange("p c (d h) -> p c d h", h=HEADS),
                            in0=glo[:, i * NL:(i + 1) * NL, 0:fdim]
                            .rearrange("p c (d h) -> p c d h", h=HEADS),
                            in1=EX[:, 0:NL].unsqueeze(2)
                            .broadcast_to([128, NL, D, HEADS]),
                            op=mybir.AluOpType.mult)
                        nc.vector.tensor_tensor(
                            out=R[:, NL:CH, 0:fdim]
                            .rearrange("p c (d h) -> p c d h", h=HEADS),
                            in0=ghi[:, i * NH:(i + 1) * NH, 0:fdim]
                            .rearrange("p c (d h) -> p c d h", h=HEADS),
                            in1=EX[:, NL:CH].unsqueeze(2)
                            .broadcast_to([128, NH, D, HEADS]),
                            op=mybir.AluOpType.mult)
                        if layer == 0:
                            # fused denominator columns
                            nc.vector.tensor_copy(
                                R[:, :, fdim:fdim + 8], EX[:])
                            P1 = pp2.tile([128, 136], F32, tag="P1")
                            for ch in range(CH):
                                nc.tensor.matmul(
                                    P1[:], inc[:, :, ch], R[:, ch, :],
                                    start=(ch == 0), stop=(ch == CH - 1))
                            post_fn(t, P1, None, op, pp2)
                        else:
                            P1 = pp2.tile([128, 512], F32, tag="P1")
                            for ch in range(CH):
                                nc.tensor.matmul(
                                    P1[:], inc[:, :, ch], R[:, ch, :],
                                    start=(ch == 0), stop=(ch == CH - 1))
                            P2 = pp2.tile([128, 8], F32, tag="P2")
                            for ch in range(CH):
                                nc.tensor.matmul(
                                    P2[:], inc[:, :, ch], EX[:, ch, :],
                                    start=(ch == 0), stop=(ch == CH - 1))
                            post_fn(t, P1, P2, op, pp2)
                        if agin_hook is not None:
                            agin_hook(t)

        # ---- L0 post: softmax-div, +b0, ELU, transpose, f8, store --------
        def post0(t, P1, _, op, pp2):
            rows = 128 if t < LT - 1 else LAST_ROWS
            k = next(kk for kk in range(NCHUNK) if t < AGB[kk + 1])
            col0 = (t - AGB[k]) * 128
            r8 = op.tile([128, 8], F32, tag="r8")
            nc.vector.tensor_scalar_add(r8[:], P1[:, 128:136], 1e-16)
            nc.vector.reciprocal(r8[:], r8[:])
            z = op.tile([128, NHID], F32, tag="z")
            nc.vector.tensor_tensor(
                out=z[:].rearrange("p (d h) -> p d h", h=HEADS),
                in0=P1[:, 0:128].rearrange("p (d h) -> p d h", h=HEADS),
                in1=r8[:].unsqueeze(1).broadcast_to([128, 16, HEADS]),
                op=mybir.AluOpType.mult)
            nc.vector.tensor_tensor(out=z[:], in0=z[:], in1=b0s[:],
                                    op=mybir.AluOpType.add)
            zm = op.tile([128, NHID], F32, tag="zm")
            nc.vector.tensor_scalar_min(zm[:], z[:], 0.0)
            nc.scalar.activation(zm[:], zm[:],
                                 mybir.ActivationFunctionType.Exp)
            zp = op.tile([128, NHID], F32, tag="zp")
            nc.vector.tensor_scalar_max(zp[:], z[:], 0.0)
            nc.vector.tensor_tensor(out=zp[:], in0=zp[:], in1=zm[:],
                                    op=mybir.AluOpType.add)
            h1 = op.tile([128, NHID], F16, tag="h1")
            nc.vector.tensor_scalar_add(h1[:], zp[:], -1.0)
            pst = pp2.tile([128, 128], F16, tag="pst")
            nc.tensor.transpose(pst[:], h1[:], idents[:])
            h8 = op.tile([128, 128], AGDT, tag="h8")
            if AG_F8:
                nc.vector.tensor_copy(h8[:].bitcast(F8), pst[:])
            else:
                nc.vector.tensor_copy(h8[:], pst[:])
            nc.sync.dma_start(
                out=agin[k][:, col0:col0 + rows], in_=h8[:, 0:rows])

        # AllGather chunks fire as soon as their agin columns are complete
        def agin_hook(t):
            for k in range(NCHUNK):
                if t == AGB[k + 1] - 1:
                    nc.gpsimd.collective_compute(
                        "AllGather", mybir.AluOpType.bypass,
                        replica_groups=[list(range(NCORES))],
                        ins=[agin[k][:]], outs=[agout[k][:]])

        if "B" in phases:
            edge_phase(0, STB, t0lo, t0hi, t0ad, NHID, post0,
                       agin_hook if "C" in phases else None)

        pid = nc.partition_id(engines=[mybir.EngineType.SP])
        sregs = [nc.sync.snap(((j + pid) % NCORES) * 128)
                 for j in range(NCORES)]
        # ---------------- phase D: layer-1 tables -------------------------
        ngt = (N + 127) // 128   # 391
        nsd = (ngt + SD - 1) // SD if "D" in phases else 0
        sts = []
        for si in range(nsd):
            r0 = si * SD * 128
            r1 = min(r0 + SD * 128, ngt * 128)
            r1c = min(r1, N)
            dep = -1
            r = r0
            while r < r1c:
                j = r // NLOC
                cl = r - j * NLOC
                k = next(kk for kk in range(NCHUNK) if cl < CHB[kk + 1])
                end = min(r1c, j * NLOC + CHB[k + 1])
                if j > 0:
                    dep = max(dep, k)
                r = end
            sts.append((dep, si, r0, r1, r1c))
        sts.sort()

        with ExitStack() as pd:
            xp1 = pd.enter_context(tc.tile_pool(name="pd_x", bufs=3))
            pp1 = pd.enter_context(
                tc.tile_pool(name="pd_ps", bufs=2, space="PSUM"))
            rp1 = pd.enter_context(tc.tile_pool(name="pd_row", bufs=3))
            for _, si, r0, r1, r1c in sts:
                nt = (r1 - r0) // 128
                hx = xp1.tile([128, SD * 128], AGDT, tag="hx")
                r = r0
                while r < r1c:
                    j = r // NLOC
                    cl = r - j * NLOC
                    k = next(kk for kk in range(NCHUNK) if cl < CHB[kk + 1])
                    end = min(r1c, j * NLOC + CHB[k + 1])
                    seg = end - r
                    if j == 0:
                        nc.sync.dma_start(
                            out=hx[:, r - r0:r - r0 + seg],
                            in_=agin[k][:, cl - CHB[k]:cl - CHB[k] + seg])
                    else:
                        nc.sync.dma_start(
                            out=hx[:, r - r0:r - r0 + seg],
                            in_=agout[k][bass.ds(sregs[j], 128),
                                         cl - CHB[k]:cl - CHB[k] + seg])
                    r = end
                if r1c < r1:
                    nc.vector.memset(hx[:, r1c - r0:r1 - r0], 0)
                row = rp1.tile([128, SD, 520], F16, tag="row")
                for g2 in range(nt):
                    hxs = hx[:, g2 * 128:(g2 + 1) * 128]
                    if AG_F8:
                        hxs = hxs.bitcast(F8)
                    ps = pp1.tile([128, 512], F32, tag=f"ps{g2 % 2}")
                    nc.tensor.matmul(ps[:], hxs, W1s[:],
                                     start=True, stop=True)
                    psa = pp1.tile([128, 16], F32, tag=f"psa{g2 % 2}")
                    nc.tensor.matmul(psa[:], hxs, W1as[:],
                                     start=True, stop=True)
                    if g2 % 2:
                        nc.scalar.copy(row[:, g2, 0:512], ps[:])
                    else:
                        nc.vector.tensor_copy(row[:, g2, 0:512], ps[:])
                    nc.vector.tensor_copy(row[:, g2, 512:520],
                                          psa[:, 0:8])
                    g = si * SD + g2
                    if g < LT:
                        nc.vector.tensor_copy(
                            adbal2[:, g * 8:(g + 1) * 8], psa[:, 8:16])
                weng = nc.scalar if si % 2 else nc.sync
                if r1 <= SPLIT:
                    weng.dma_start(
                        out=t1lo[r0:r1, 0:520]
                        .rearrange("(g p) w -> p g w", p=128),
                        in_=row[:, 0:nt, :])
                elif r0 >= SPLIT:
                    weng.dma_start(
                        out=t1hi[r0 - SPLIT:r1 - SPLIT, 0:520]
                        .rearrange("(g p) w -> p g w", p=128),
                        in_=row[:, 0:nt, :])
                else:
                    nlo = (SPLIT - r0) // 128
                    weng.dma_start(
                        out=t1lo[r0:SPLIT, 0:520]
                        .rearrange("(g p) w -> p g w", p=128),
                        in_=row[:, 0:nlo, :])
                    weng.dma_start(
                        out=t1hi[0:r1 - SPLIT, 0:520]
                        .rearrange("(g p) w -> p g w", p=128),
                        in_=row[:, nlo:nt, :])
            if "D" in phases:
                nc.sync.dma_start(
                    out=t1ad[:, 0:8].rearrange("(t p) w -> p t w", p=128),
                    in_=adbal2[:].rearrange("p (t w) -> p t w", w=8))

        # ---------------- phase E: layer-1 edges + epilogue ---------------
        fpool = stk.enter_context(tc.tile_pool(name="fin", bufs=1))
        zbig = fpool.tile([128, LT * NCLASS], F32)
        nmxb = fpool.tile([128, LT], F32)
        seb = fpool.tile([128, LT], F32)

        def post1(t, P1, P2, op, pp2):
            r8 = op.tile([128, 8], F32, tag="r8")
            nc.vector.tensor_scalar_add(r8[:], P2[:], 1e-16)
            nc.vector.reciprocal(r8[:], r8[:])
            nc.vector.tensor_scalar_mul(r8[:], r8[:], 1.0 / HEADS)
            zw = op.tile([128, 512], F32, tag="zw")
            nc.vector.tensor_tensor(
                out=zw[:].rearrange("p (d h) -> p d h", h=HEADS),
                in0=P1[:].rearrange("p (d h) -> p d h", h=HEADS),
                in1=r8[:].unsqueeze(1).broadcast_to([128, 64, HEADS]),
                op=mybir.AluOpType.mult)
            z = zbig[:, t * NCLASS:(t + 1) * NCLASS]
            nc.vector.reduce_sum(
                z, zw[:].rearrange("p (d h) -> p d h", h=HEADS),
                axis=mybir.AxisListType.X)
            nc.vector.tensor_tensor(out=z, in0=z, in1=b1s[:],
                                    op=mybir.AluOpType.add)
            nmx = nmxb[:, t:t + 1]
            nc.vector.reduce_max(nmx, z, axis=mybir.AxisListType.X,
                                 negate=True)
            ez = op.tile([128, NCLASS], F32, tag="ez")
            nc.scalar.activation(ez[:], z,
                                 mybir.ActivationFunctionType.Exp,
                                 bias=nmx, accum_out=seb[:, t:t + 1])

        if "E" in phases:
            edge_phase(1, STE, t1lo, t1hi, t1ad, 512, post1, None)
        else:
            nc.vector.memset(zbig[:], 0)
            nc.vector.memset(nmxb[:], 0)
            nc.vector.memset(seb[:], 1.0)

        # batched log-softmax tail
        nc.scalar.activation(seb[:], seb[:],
                             mybir.ActivationFunctionType.Ln)
        nc.vector.tensor_tensor(
            out=zbig[:].rearrange("p (t c) -> p t c", c=NCLASS),
            in0=zbig[:].rearrange("p (t c) -> p t c", c=NCLASS),
            in1=nmxb[:].unsqueeze(-1).broadcast_to([128, LT, NCLASS]),
            op=mybir.AluOpType.add)
        nc.vector.tensor_tensor(
            out=zbig[:].rearrange("p (t c) -> p t c", c=NCLASS),
            in0=zbig[:].rearrange("p (t c) -> p t c", c=NCLASS),
            in1=seb[:].unsqueeze(-1).broadcast_to([128, LT, NCLASS]),
            op=mybir.AluOpType.subtract)
        nfull = (LT - 1) * 128
        nc.sync.dma_start(
            out=out[0:nfull, :].rearrange("(t p) c -> p t c", p=128),
            in_=zbig[:].rearrange("p (t c) -> p t c", c=NCLASS)
            [:, 0:LT - 1, :])
        nc.sync.dma_start(
            out=out[nfull:nfull + LAST_ROWS, :],
            in_=zbig[0:LAST_ROWS, (LT - 1) * NCLASS:LT * NCLASS])

    nc.compile()
    return nc


# --------------------------------------------------------------------------
# entry point
# --------------------------------------------------------------------------

def kernel(**inputs) -> np.ndarray:
    NLk, NHk, in_maps = _prep_inputs(**inputs)
    key = (NLk, NHk)
    if key not in _cache:
        _cache[key] = build(NLk, NHk)
    nc = _cache[key]
    res = run_bass_kernel_spmd(nc, in_maps, list(range(NCORES)))
    return np.concatenate([res.results[c]["out"] for c in range(NCORES)], 0)


# revision 18
# speedup vs baseline: 1.0034x; 1.0034x over previous
"""2-layer GAT (nn_GAT_31490700214331) on 8 Trainium2 NeuronCores.

Strategy (dst-sharded, SPMD, per-core-rotated node layout) — v2:
  - Nodes block-partitioned: core c owns nodes [c*6250, (c+1)*6250); every
    table on core c uses a ROTATED row order (node n at row (n - c*6250)
    mod 50000) so one SPMD program serves all cores.
  - Phase A (replicated): h0 = x @ [W0 | W0·a_src | W0·a_dst] for all
    nodes; rows [h0|as0] land in the gather tables t0lo/t0hi (512 B rows),
    dst-alphas accumulate in SBUF and are written to t0ad in one DMA.
  - Phase B: per group of 4 dst tiles, three batched dma_gathers (src rows
    lo/hi + per-edge dst-alpha); edge softmax (safe without segment-max)
    and aggregation run as 128x128 incidence matmuls; denominators ride
    as 8 fused psum columns.  Incidence builds alternate DVE/GpSimd.
  - The hidden state is ELU'd, transposed, quantized to f8e4 and
    AllGather'd in 4 column chunks that overlap phase B's tail and
    phase D's head (COLLECTIVE_CORES runs concurrently with compute).
  - Phase D: supertiles of 8 node tiles, ordered by which AllGather chunk
    they need (own-core rows first, straight from local agin); the f8
    hidden state feeds mixed-precision matmuls with W1/W1a; rows
    [h1|as1] go to t1lo/t1hi (1280 B rows), dst-alphas to t1ad.
  - Phase E: like B with 640-col rows, separate denominator chain, and a
    head-mean + batched log_softmax epilogue.
  - alpha projections fold into the weight matmuls on the host:
    h @ blockdiag(a) == x @ (W @ blockdiag(a)).

Self-contained: call kernel(**inputs) with the full-problem arrays.
"""
import numpy as np
from contextlib import ExitStack

import concourse.bacc as bacc
import concourse.bass as bass
import concourse.mybir as mybir
from concourse.tile import TileContext
from concourse.bass_utils import run_bass_kernel_spmd

F16 = mybir.dt.float16
F32 = mybir.dt.float32
F8 = mybir.dt.float8e4
I16 = mybir.dt.int16
I8 = mybir.dt.int8

N = 50000
NFEAT = 256
NHID = 128
NCLASS = 64
HEADS = 8
SLOPE = 0.2
NCORES = 8
NLOC = N // NCORES           # 6250
LT = (NLOC + 127) // 128     # 49 local dst tiles
LAST_ROWS = NLOC - (LT - 1) * 128   # 106 rows in the last tile
GT = 392                     # global node tiles (392*128 = 50176)
GROWS = GT * 128
SPLIT = 25088                # low/high gather-table split (196 tiles)
SENT = 300.0                 # dst_rel sentinel for padding slots
T0W = 256                    # t0 row: [h0(128)|as0(8)|junk] f16
T1W = 640                    # t1 row: [h1(512)|as1(8)|junk] f16
STB = 4                      # layer-0 gather supertile (dst tiles)
STE = 2                      # layer-1 gather supertile
SD = 8                       # phase-D node tiles per supertile
NCHUNK = 4                   # AllGather chunks
AGCH = {1: [49], 2: [25, 24], 4: [12, 12, 12, 13]}[NCHUNK]
AGB = list(np.cumsum([0] + AGCH))          # tile boundaries
CHB = [min(b * 128, NLOC) for b in AGB]    # col boundaries
AG_F8 = False                # f8 AllGather payload (else f16)
GBATCH = False               # multi-tile gathers (SWDGE ring risk on HW)
AGLAG = 6                    # tiles of slack before an AllGather is queued

_cache = {}


# --------------------------------------------------------------------------
# host-side preparation
# --------------------------------------------------------------------------

def _wrap_idx(idx):
    """[n] int -> [128, n//16] int16 wrapped gather-index layout."""
    n = idx.shape[0]
    assert n % 16 == 0
    w = idx.reshape(n // 16, 16).T.astype(np.int16)
    return np.tile(w, (8, 1))


def _prep_edges(src, dst):
    cores = []
    for c in range(NCORES):
        m = (dst >= c * NLOC) & (dst < (c + 1) * NLOC)
        s = src[m].astype(np.int64)
        d = dst[m].astype(np.int64) - c * NLOC
        order = np.argsort(d, kind="stable")
        s, d = s[order], d[order]
        s_rot = (s - c * NLOC) % N
        tiles = []
        for t in range(LT):
            sel = (d >= t * 128) & (d < (t + 1) * 128)
            st, dt = s_rot[sel], d[sel] - t * 128
            lo = st < SPLIT
            tiles.append((st[lo], dt[lo], st[~lo] - SPLIT, dt[~lo]))
        cores.append(tiles)
    nl = max(len(t[0]) for tl in cores for t in tl)
    nh = max(len(t[2]) for tl in cores for t in tl)
    NL = max(1, (nl + 127) // 128)
    NH = max(1, (nh + 127) // 128)
    CH = NL + NH

    out = []
    for c in range(NCORES):
        ilb = np.zeros((LT, 128, NL * 8), np.int16)
        ihb = np.zeros((LT, 128, NH * 8), np.int16)
        aib = np.zeros((LT, 128, CH * 8), np.int16)
        drb = np.zeros((LT, 128, CH), np.int16)
        for t in range(LT):
            sl, dl, sh, dh = cores[c][t]
            il = np.zeros(NL * 128, np.int64)
            il[: len(sl)] = sl
            ih = np.zeros(NH * 128, np.int64)
            ih[: len(sh)] = sh
            ai = np.zeros(CH * 128, np.int64)
            ai[: len(dl)] = t * 128 + dl
            ai[NL * 128: NL * 128 + len(dh)] = t * 128 + dh
            ilb[t] = _wrap_idx(il)
            ihb[t] = _wrap_idx(ih)
            aib[t] = _wrap_idx(ai)
            rl = np.full(NL * 128, SENT)
            rl[: len(dl)] = dl
            rh = np.full(NH * 128, SENT)
            rh[: len(dh)] = dh
            r = np.concatenate([rl, rh]).reshape(CH, 128).T
            drb[t] = r.astype(np.float16).view(np.int16)
        epack = np.concatenate(
            [ilb.transpose(1, 0, 2).reshape(128, -1),
             ihb.transpose(1, 0, 2).reshape(128, -1),
             aib.transpose(1, 0, 2).reshape(128, -1),
             drb.transpose(1, 0, 2).reshape(128, -1)], axis=1)
        out.append(dict(epack=np.ascontiguousarray(epack)))
    return NL, NH, out


def _prep_inputs(x, edge_index, W0, a_src0, a_dst0, b0, W1, a_src1, a_dst1,
                 b1):
    src = np.asarray(edge_index[0]).astype(np.int64)
    dst = np.asarray(edge_index[1]).astype(np.int64)
    NL, NH, edata = _prep_edges(src, dst)

    def bd(a):  # [H, D] -> blockdiag [H*D, H]
        a = np.asarray(a, np.float32)
        H, D = a.shape
        m = np.zeros((H * D, H), np.float32)
        for h in range(H):
            m[h * D:(h + 1) * D, h] = a[h]
        return m

    W0 = np.asarray(W0, np.float32)
    W1 = np.asarray(W1, np.float32)
    W0a = np.concatenate([W0 @ bd(a_src0), W0 @ bd(a_dst0)], 1)  # [256, 16]
    # head-innermost feature interleave: new col d*8+h <- old col h*D+d
    perm0 = np.array([(f % 8) * 16 + f // 8 for f in range(128)])
    perm1 = np.array([(f % 8) * 64 + f // 8 for f in range(512)])
    W0cat = np.concatenate([W0[:, perm0], W0a], 1)               # [256, 144]
    W1a = np.concatenate([W1 @ bd(a_src1), W1 @ bd(a_dst1)], 1)  # [128, 16]

    x = np.asarray(x, np.float32)
    ident = np.eye(128, dtype=np.float16)
    CH = NL + NH
    colio = np.tile(np.repeat(np.arange(128, dtype=np.float16), CH)[None, :],
                    (128, 1))
    b0b = np.tile(np.asarray(b0, np.float32)[None, :], (128, 1))
    b1b = np.tile(np.asarray(b1, np.float32)[None, :], (128, 1))

    in_maps = []
    for c in range(NCORES):
        rot = np.roll(np.arange(N), -c * NLOC)
        xr = np.zeros((GROWS, NFEAT), np.float16)
        xr[:N] = x[rot].astype(np.float16)
        xtt = xr.reshape(GROWS // 128, 128, 2, 128).transpose(0, 3, 2, 1)
        m = dict(
            xT=np.ascontiguousarray(xtt),
            W0=np.ascontiguousarray(
                W0cat.astype(np.float16).reshape(2, 128, NHID + 16)),
            W1=np.ascontiguousarray(W1[perm0][:, perm1].astype(np.float16)),
            W1a=np.ascontiguousarray(W1a[perm0].astype(np.float16)),
            b0b=np.ascontiguousarray(b0b[:, perm0]), b1b=b1b,
            ident=ident, colio=colio,
            **edata[c],
        )
        in_maps.append(m)
    return NL, NH, in_maps


# --------------------------------------------------------------------------
# device program
# --------------------------------------------------------------------------

def build(NL, NH, phases="ABCDE"):
    CH = NL + NH
    HID16 = NHID + 16
    ILB = 0                       # epack col offsets (int16 cols)
    IHB = ILB + LT * NL * 8
    AIB = IHB + LT * NH * 8
    DRB = AIB + LT * CH * 8
    TOT = DRB + LT * CH

    nc = bacc.Bacc("TRN2")
    xT = nc.dram_tensor("xT", [GT, 128, 2, 128], F16, kind="ExternalInput")
    W0i = nc.dram_tensor("W0", [2, 128, HID16], F16, kind="ExternalInput")
    W1i = nc.dram_tensor("W1", [NHID, 512], F16, kind="ExternalInput")
    W1ai = nc.dram_tensor("W1a", [NHID, 16], F16, kind="ExternalInput")
    b0bi = nc.dram_tensor("b0b", [128, NHID], F32, kind="ExternalInput")
    b1bi = nc.dram_tensor("b1b", [128, NCLASS], F32, kind="ExternalInput")
    identi = nc.dram_tensor("ident", [128, 128], F16, kind="ExternalInput")
    colioi = nc.dram_tensor("colio", [128, 128 * CH], F16,
                            kind="ExternalInput")
    epacki = nc.dram_tensor("epack", [128, TOT], I16, kind="ExternalInput")
    out = nc.dram_tensor("out", [NLOC, NCLASS], F32, kind="ExternalOutput")

    with TileContext(nc) as tc, ExitStack() as stk:
        dpool = stk.enter_context(
            tc.tile_pool(name="dram", bufs=1, space="DRAM"))
        t0lo = dpool.tile([SPLIT, T0W], F16, tag="t0lo")
        t0hi = dpool.tile([GROWS - SPLIT, T0W], F16, tag="t0hi")
        t0ad = dpool.tile([LT * 128, 128], F16, tag="t0ad")
        t1lo = dpool.tile([SPLIT, T1W], F16, tag="t1lo")
        t1hi = dpool.tile([GROWS - SPLIT, T1W], F16, tag="t1hi")
        t1ad = dpool.tile([LT * 128, 128], F16, tag="t1ad")
        AGDT = I8 if AG_F8 else F16
        agin = []
        agout = []
        for k in range(NCHUNK):
            agin_k = dpool.tile([128, AGCH[k] * 128], AGDT, tag=f"agin{k}",
                                name=f"agin{k}")
            agout_k = dpool.tile([NCORES * 128, AGCH[k] * 128], AGDT,
                                 tag=f"agout{k}", addr_space="Shared",
                                 name=f"agout{k}")
            agin.append(agin_k)
            agout.append(agout_k)

        cpool = stk.enter_context(tc.tile_pool(name="const", bufs=1))
        W0s = cpool.tile([128, 2, HID16], F16)
        nc.sync.dma_start(out=W0s[:], in_=W0i.rearrange("k p n -> p k n"))
        W1s = cpool.tile([128, 512], F16)
        nc.sync.dma_start(out=W1s[:], in_=W1i[:])
        W1as = cpool.tile([128, 16], F16)
        nc.sync.dma_start(out=W1as[:], in_=W1ai[:])
        b0s = cpool.tile([128, NHID], F32)
        nc.sync.dma_start(out=b0s[:], in_=b0bi[:])
        b1s = cpool.tile([128, NCLASS], F32)
        nc.sync.dma_start(out=b1s[:], in_=b1bi[:])
        idents = cpool.tile([128, 128], F16)
        nc.sync.dma_start(out=idents[:], in_=identi[:])
        colios = cpool.tile([128, 128 * CH], F16)
        nc.sync.dma_start(out=colios[:], in_=colioi[:])
        adball = cpool.tile([128, LT * 8], F16)
        adbal2 = cpool.tile([128, LT * 8], F16)
        epS = cpool.tile([128, TOT], I16)
        nc.scalar.dma_start(out=epS[:], in_=epacki[:])

        regs = {}

        def reg(n):
            if n not in regs:
                regs[n] = nc.gpsimd.to_reg(n)
            return regs[n]

        # ---------------- phase A: layer-0 tables (replicated) ------------
        with ExitStack() as pa:
            xp = pa.enter_context(tc.tile_pool(name="pa_x", bufs=3))
            pp = pa.enter_context(
                tc.tile_pool(name="pa_ps", bufs=1, space="PSUM"))
            rp = pa.enter_context(tc.tile_pool(name="pa_row", bufs=3))
            for gg in range(GT // 8):
                xa = xp.tile([128, 8, 2, 128], F16, tag="xa")
                eng = nc.sync if gg % 2 else nc.scalar
                eng.dma_start(
                    out=xa[:],
                    in_=xT[8 * gg:8 * gg + 8].rearrange(
                        "g p k j -> p g k j"))
                row = rp.tile([128, 8, 136], F16, tag="row")
                for g2 in range(8):
                    ps = pp.tile([128, HID16], F32, tag=f"ps{g2 % 4}")
                    for k in range(2):
                        nc.tensor.matmul(ps[:], xa[:, g2, k, :],
                                         W0s[:, k, :],
                                         start=(k == 0), stop=(k == 1))
                    nc.vector.tensor_copy(row[:, g2, :], ps[:, 0:136])
                    g = 8 * gg + g2
                    if g < LT:
                        nc.vector.tensor_copy(
                            adball[:, g * 8:(g + 1) * 8], ps[:, 136:144])
                g0 = gg * 1024
                weng = nc.scalar if gg % 2 else nc.sync
                if g0 + 1024 <= SPLIT:
                    weng.dma_start(
                        out=t0lo[g0:g0 + 1024, 0:136]
                        .rearrange("(g p) w -> p g w", p=128),
                        in_=row[:])
                elif g0 >= SPLIT:
                    o = g0 - SPLIT
                    weng.dma_start(
                        out=t0hi[o:o + 1024, 0:136]
                        .rearrange("(g p) w -> p g w", p=128),
                        in_=row[:])
                else:  # straddles the split (gg == 24)
                    nlo = (SPLIT - g0) // 128
                    weng.dma_start(
                        out=t0lo[g0:SPLIT, 0:136]
                        .rearrange("(g p) w -> p g w", p=128),
                        in_=row[:, 0:nlo])
                    weng.dma_start(
                        out=t0hi[0:1024 - (SPLIT - g0), 0:136]
                        .rearrange("(g p) w -> p g w", p=128),
                        in_=row[:, nlo:8])
            nc.sync.dma_start(
                out=t0ad[:, 0:8].rearrange("(t p) w -> p t w", p=128),
                in_=adball[:].rearrange("p (t w) -> p t w", w=8))

        # ---------------- shared edge phase -------------------------------
        def edge_phase(layer, ST, tbl_lo, tbl_hi, tblad, fdim, post_fn,
                       agin_hook):
            D = fdim // HEADS
            trow = T1W if layer else T0W
            nst = (LT + ST - 1) // ST
            with ExitStack() as pb:
                gp = pb.enter_context(
                    tc.tile_pool(name=f"gg{layer}", bufs=3 if layer == 0
                                 else 2))
                apl = pb.enter_context(
                    tc.tile_pool(name=f"ga{layer}", bufs=3 if layer == 0
                                 else 2))
                incp = pb.enter_context(
                    tc.tile_pool(name=f"ic{layer}", bufs=3))
                rp2 = pb.enter_context(
                    tc.tile_pool(name=f"rh{layer}", bufs=3))
                exq = pb.enter_context(
                    tc.tile_pool(name=f"ex{layer}", bufs=3))
                pp2 = pb.enter_context(
                    tc.tile_pool(name=f"ps{layer}", bufs=3, space="PSUM"))
                op = pb.enter_context(
                    tc.tile_pool(name=f"po{layer}", bufs=3))
                for st in range(nst):
                    a, b = ST * st, min(ST * st + ST, LT)
                    nt = b - a
                    glo = gp.tile([128, ST * NL, trow], F16, tag="glo")
                    ghi = gp.tile([128, ST * NH, trow], F16, tag="ghi")
                    ga = apl.tile([128, ST * CH, 128], F16, tag="ga")
                    if GBATCH:
                        nc.gpsimd.dma_gather(
                            glo[:, 0:nt * NL, :], tbl_lo[:],
                            epS[:, ILB + a * NL * 8:ILB + b * NL * 8],
                            nt * NL * 128, reg(nt * NL * 128), trow)
                        nc.gpsimd.dma_gather(
                            ghi[:, 0:nt * NH, :], tbl_hi[:],
                            epS[:, IHB + a * NH * 8:IHB + b * NH * 8],
                            nt * NH * 128, reg(nt * NH * 128), trow)
                        nc.gpsimd.dma_gather(
                            ga[:, 0:nt * CH, :], tblad[:],
                            epS[:, AIB + a * CH * 8:AIB + b * CH * 8],
                            nt * CH * 128, reg(nt * CH * 128), 128)
                    else:
                        for t in range(a, b):
                            i = t - a
                            nc.gpsimd.dma_gather(
                                glo[:, i * NL:(i + 1) * NL, :], tbl_lo[:],
                                epS[:, ILB + t * NL * 8:
                                    ILB + (t + 1) * NL * 8],
                                NL * 128, reg(NL * 128), trow)
                            nc.gpsimd.dma_gather(
                                ghi[:, i * NH:(i + 1) * NH, :], tbl_hi[:],
                                epS[:, IHB + t * NH * 8:
                                    IHB + (t + 1) * NH * 8],
                                NH * 128, reg(NH * 128), trow)
                            nc.gpsimd.dma_gather(
                                ga[:, i * CH:i * CH + NL, :], tblad[:],
                                epS[:, AIB + t * CH * 8:
                                    AIB + t * CH * 8 + NL * 8],
                                NL * 128, reg(NL * 128), 128)
                            nc.gpsimd.dma_gather(
                                ga[:, i * CH + NL:(i + 1) * CH, :],
                                tblad[:],
                                epS[:, AIB + t * CH * 8 + NL * 8:
                                    AIB + (t + 1) * CH * 8],
                                NH * 128, reg(NH * 128), 128)
                    for t in range(a, b):
                        i = t - a
                        dr = epS[:, DRB + t * CH:DRB + (t + 1) * CH]\
                            .bitcast(F16)
                        inc = incp.tile([128, 128, CH], F16, tag="inc")
                        nc.vector.tensor_tensor(
                            out=inc[:],
                            in0=dr.unsqueeze(1)
                            .broadcast_to([128, 128, CH]),
                            in1=colios[:]
                            .rearrange("p (d c) -> p d c", c=CH),
                            op=mybir.AluOpType.is_equal)
                        EX = exq.tile([128, CH, 8], F16, tag="EX")
                        nc.vector.tensor_tensor(
                            out=EX[:, 0:NL, :],
                            in0=glo[:, i * NL:(i + 1) * NL,
                                    fdim:fdim + 8],
                            in1=ga[:, i * CH:i * CH + NL, 0:8],
                            op=mybir.AluOpType.add)
                        nc.vector.tensor_tensor(
                            out=EX[:, NL:CH, :],
                            in0=ghi[:, i * NH:(i + 1) * NH,
                                    fdim:fdim + 8],
                            in1=ga[:, i * CH + NL:(i + 1) * CH, 0:8],
                            op=mybir.AluOpType.add)
                        nc.scalar.activation(
                            EX[:], EX[:],
                            mybir.ActivationFunctionType.Prelu,
                            alpha=SLOPE)
                        nc.scalar.activation(
                            EX[:], EX[:],
                            mybir.ActivationFunctionType.Exp)
                        rw = fdim + 8 if layer == 0 else fdim
                        R = rp2.tile([128, CH, rw], F16, tag="R")
                        nc.vector.tensor_tensor(
                            out=R[:, 0:NL, 0:fdim]
                            .rearrange("p c (d h) -> p c d h", h=HEADS),
                            in0=glo[:, i * NL:(i + 1) * NL, 0:fdim]
                            .rearrange("p c (d h) -> p c d h", h=HEADS),
                            in1=EX[:, 0:NL].unsqueeze(2)
                            .broadcast_to([128, NL, D, HEADS]),
                            op=mybir.AluOpType.mult)
                        nc.vector.tensor_tensor(
                            out=R[:, NL:CH, 0:fdim]
                            .rearrange("p c (d h) -> p c d h", h=HEADS),
                            in0=ghi[:, i * NH:(i + 1) * NH, 0:fdim]
                            .rearrange("p c (d h) -> p c d h", h=HEADS),
                            in1=EX[:, NL:CH].unsqueeze(2)
                            .broadcast_to([128, NH, D, HEADS]),
                            op=mybir.AluOpType.mult)
                        if layer == 0:
                            # fused denominator columns
                            nc.vector.tensor_copy(
                                R[:, :, fdim:fdim + 8], EX[:])
                            P1 = pp2.tile([128, 136], F32, tag="P1")
                            for ch in range(CH):
                                nc.tensor.matmul(
                                    P1[:], inc[:, :, ch], R[:, ch, :],
                                    start=(ch == 0), stop=(ch == CH - 1))
                            post_fn(t, P1, None, op, pp2)
                        else:
                            P1 = pp2.tile([128, 512], F32, tag="P1")
                            for ch in range(CH):
                                nc.tensor.matmul(
                                    P1[:], inc[:, :, ch], R[:, ch, :],
                                    start=(ch == 0), stop=(ch == CH - 1))
                            P2 = pp2.tile([128, 8], F32, tag="P2")
                            for ch in range(CH):
                                nc.tensor.matmul(
                                    P2[:], inc[:, :, ch], EX[:, ch, :],
                                    start=(ch == 0), stop=(ch == CH - 1))
                            post_fn(t, P1, P2, op, pp2)
                        if agin_hook is not None:
                            agin_hook(t)

        # ---- L0 post: softmax-div, +b0, ELU, transpose, f8, store --------
        def post0(t, P1, _, op, pp2):
            rows = 128 if t < LT - 1 else LAST_ROWS
            k = next(kk for kk in range(NCHUNK) if t < AGB[kk + 1])
            col0 = (t - AGB[k]) * 128
            r8 = op.tile([128, 8], F32, tag="r8")
            nc.vector.tensor_scalar_add(r8[:], P1[:, 128:136], 1e-16)
            nc.vector.reciprocal(r8[:], r8[:])
            z = op.tile([128, NHID], F32, tag="z")
            nc.vector.tensor_tensor(
                out=z[:].rearrange("p (d h) -> p d h", h=HEADS),
                in0=P1[:, 0:128].rearrange("p (d h) -> p d h", h=HEADS),
                in1=r8[:].unsqueeze(1).broadcast_to([128, 16, HEADS]),
                op=mybir.AluOpType.mult)
            nc.vector.tensor_tensor(out=z[:], in0=z[:], in1=b0s[:],
                                    op=mybir.AluOpType.add)
            zm = op.tile([128, NHID], F32, tag="zm")
            nc.vector.tensor_scalar_min(zm[:], z[:], 0.0)
            nc.scalar.activation(zm[:], zm[:],
                                 mybir.ActivationFunctionType.Exp)
            zp = op.tile([128, NHID], F32, tag="zp")
            nc.vector.tensor_scalar_max(zp[:], z[:], 0.0)
            nc.vector.tensor_tensor(out=zp[:], in0=zp[:], in1=zm[:],
                                    op=mybir.AluOpType.add)
            h1 = op.tile([128, NHID], F16, tag="h1")
            nc.vector.tensor_scalar_add(h1[:], zp[:], -1.0)
            pst = pp2.tile([128, 128], F16, tag="pst")
            nc.tensor.transpose(pst[:], h1[:], idents[:])
            h8 = op.tile([128, 128], AGDT, tag="h8")
            if AG_F8:
                nc.vector.tensor_copy(h8[:].bitcast(F8), pst[:])
            else:
                nc.vector.tensor_copy(h8[:], pst[:])
            nc.sync.dma_start(
                out=agin[k][:, col0:col0 + rows], in_=h8[:, 0:rows])

        # AllGather chunks fire as soon as their agin columns are complete
        def agin_hook(t):
            for k in range(NCHUNK):
                if t == min(AGB[k + 1] - 1 + AGLAG, LT - 1) and (
                        k == NCHUNK - 1 or AGB[k + 1] - 1 + AGLAG < LT - 1):
                    nc.gpsimd.collective_compute(
                        "AllGather", mybir.AluOpType.bypass,
                        replica_groups=[list(range(NCORES))],
                        ins=[agin[k][:]], outs=[agout[k][:]])

        if "B" in phases:
            edge_phase(0, STB, t0lo, t0hi, t0ad, NHID, post0,
                       agin_hook if "C" in phases else None)

        pid = nc.partition_id(engines=[mybir.EngineType.SP])
        sregs = [nc.sync.snap(((j + pid) % NCORES) * 128)
                 for j in range(NCORES)]
        # ---------------- phase D: layer-1 tables -------------------------
        ngt = (N + 127) // 128   # 391
        nsd = (ngt + SD - 1) // SD if "D" in phases else 0
        sts = []
        for si in range(nsd):
            r0 = si * SD * 128
            r1 = min(r0 + SD * 128, ngt * 128)
            r1c = min(r1, N)
            dep = -1
            r = r0
            while r < r1c:
                j = r // NLOC
                cl = r - j * NLOC
                k = next(kk for kk in range(NCHUNK) if cl < CHB[kk + 1])
                end = min(r1c, j * NLOC + CHB[k + 1])
                if j > 0:
                    dep = max(dep, k)
                r = end
            sts.append((dep, si, r0, r1, r1c))
        sts.sort()

        with ExitStack() as pd:
            xp1 = pd.enter_context(tc.tile_pool(name="pd_x", bufs=3))
            pp1 = pd.enter_context(
                tc.tile_pool(name="pd_ps", bufs=2, space="PSUM"))
            rp1 = pd.enter_context(tc.tile_pool(name="pd_row", bufs=3))
            for _, si, r0, r1, r1c in sts:
                nt = (r1 - r0) // 128
                hx = xp1.tile([128, SD * 128], AGDT, tag="hx")
                r = r0
                while r < r1c:
                    j = r // NLOC
                    cl = r - j * NLOC
                    k = next(kk for kk in range(NCHUNK) if cl < CHB[kk + 1])
                    end = min(r1c, j * NLOC + CHB[k + 1])
                    seg = end - r
                    if j == 0:
                        nc.sync.dma_start(
                            out=hx[:, r - r0:r - r0 + seg],
                            in_=agin[k][:, cl - CHB[k]:cl - CHB[k] + seg])
                    else:
                        nc.sync.dma_start(
                            out=hx[:, r - r0:r - r0 + seg],
                            in_=agout[k][bass.ds(sregs[j], 128),
                                         cl - CHB[k]:cl - CHB[k] + seg])
                    r = end
                if r1c < r1:
                    nc.vector.memset(hx[:, r1c - r0:r1 - r0], 0)
                row = rp1.tile([128, SD, 520], F16, tag="row")
                for g2 in range(nt):
                    hxs = hx[:, g2 * 128:(g2 + 1) * 128]
                    if AG_F8:
                        hxs = hxs.bitcast(F8)
                    ps = pp1.tile([128, 512], F32, tag=f"ps{g2 % 2}")
                    nc.tensor.matmul(ps[:], hxs, W1s[:],
                                     start=True, stop=True)
                    psa = pp1.tile([128, 16], F32, tag=f"psa{g2 % 2}")
                    nc.tensor.matmul(psa[:], hxs, W1as[:],
                                     start=True, stop=True)
                    if g2 % 2:
                        nc.scalar.copy(row[:, g2, 0:512], ps[:])
                    else:
                        nc.vector.tensor_copy(row[:, g2, 0:512], ps[:])
                    nc.vector.tensor_copy(row[:, g2, 512:520],
                                          psa[:, 0:8])
                    g = si * SD + g2
                    if g < LT:
                        nc.vector.tensor_copy(
                            adbal2[:, g * 8:(g + 1) * 8], psa[:, 8:16])
                weng = nc.scalar if si % 2 else nc.sync
                if r1 <= SPLIT:
                    weng.dma_start(
                        out=t1lo[r0:r1, 0:520]
                        .rearrange("(g p) w -> p g w", p=128),
                        in_=row[:, 0:nt, :])
                elif r0 >= SPLIT:
                    weng.dma_start(
                        out=t1hi[r0 - SPLIT:r1 - SPLIT, 0:520]
                        .rearrange("(g p) w -> p g w", p=128),
                        in_=row[:, 0:nt, :])
                else:
                    nlo = (SPLIT - r0) // 128
                    weng.dma_start(
                        out=t1lo[r0:SPLIT, 0:520]
                        .rearrange("(g p) w -> p g w", p=128),
                        in_=row[:, 0:nlo, :])
                    weng.dma_start(
                        out=t1hi[0:r1 - SPLIT, 0:520]
                        .rearrange("(g p) w -> p g w", p=128),
                        in_=row[:, nlo:nt, :])
            if "D" in phases:
                nc.sync.dma_start(
                    out=t1ad[:, 0:8].rearrange("(t p) w -> p t w", p=128),
                    in_=adbal2[:].rearrange("p (t w) -> p t w", w=8))

        # ---------------- phase E: layer-1 edges + epilogue ---------------
        fpool = stk.enter_context(tc.tile_pool(name="fin", bufs=1))
        zbig = fpool.tile([128, LT * NCLASS], F32)
        nmxb = fpool.tile([128, LT], F32)
        seb = fpool.tile([128, LT], F32)

        def post1(t, P1, P2, op, pp2):
            r8 = op.tile([128, 8], F32, tag="r8")
            nc.vector.tensor_scalar_add(r8[:], P2[:], 1e-16)
            nc.vector.reciprocal(r8[:], r8[:])
            nc.vector.tensor_scalar_mul(r8[:], r8[:], 1.0 / HEADS)
            zw = op.tile([128, 512], F32, tag="zw")
            nc.vector.tensor_tensor(
                out=zw[:].rearrange("p (d h) -> p d h", h=HEADS),
                in0=P1[:].rearrange("p (d h) -> p d h", h=HEADS),
                in1=r8[:].unsqueeze(1).broadcast_to([128, 64, HEADS]),
                op=mybir.AluOpType.mult)
            z = zbig[:, t * NCLASS:(t + 1) * NCLASS]
            nc.vector.reduce_sum(
                z, zw[:].rearrange("p (d h) -> p d h", h=HEADS),
                axis=mybir.AxisListType.X)
            nc.vector.tensor_tensor(out=z, in0=z, in1=b1s[:],
                                    op=mybir.AluOpType.add)
            nmx = nmxb[:, t:t + 1]
            nc.vector.reduce_max(nmx, z, axis=mybir.AxisListType.X,
                                 negate=True)
            ez = op.tile([128, NCLASS], F32, tag="ez")
            nc.scalar.activation(ez[:], z,
                                 mybir.ActivationFunctionType.Exp,
                                 bias=nmx, accum_out=seb[:, t:t + 1])

        if "E" in phases:
            edge_phase(1, STE, t1lo, t1hi, t1ad, 512, post1, None)
        else:
            nc.vector.memset(zbig[:], 0)
            nc.vector.memset(nmxb[:], 0)
            nc.vector.memset(seb[:], 1.0)

        # batched log-softmax tail
        nc.scalar.activation(seb[:], seb[:],
                             mybir.ActivationFunctionType.Ln)
        nc.vector.tensor_tensor(
            out=zbig[:].rearrange("p (t c) -> p t c", c=NCLASS),
            in0=zbig[:].rearrange("p (t c) -> p t c", c=NCLASS),
            in1=nmxb[:].unsqueeze(-1).broadcast_to([128, LT, NCLASS]),
            op=mybir.AluOpType.add)
        nc.vector.tensor_tensor(
            out=zbig[:].rearrange("p (t c) -> p t c", c=NCLASS),
            in0=zbig[:].rearrange("p (t c) -> p t c", c=NCLASS),
            in1=seb[:].unsqueeze(-1).broadcast_to([128, LT, NCLASS]),
            op=mybir.AluOpType.subtract)
        nfull = (LT - 1) * 128
        nc.sync.dma_start(
            out=out[0:nfull, :].rearrange("(t p) c -> p t c", p=128),
            in_=zbig[:].rearrange("p (t c) -> p t c", c=NCLASS)
            [:, 0:LT - 1, :])
        nc.sync.dma_start(
            out=out[nfull:nfull + LAST_ROWS, :],
            in_=zbig[0:LAST_ROWS, (LT - 1) * NCLASS:LT * NCLASS])

    nc.compile()
    return nc


# --------------------------------------------------------------------------
# entry point
# --------------------------------------------------------------------------

def kernel(**inputs) -> np.ndarray:
    NLk, NHk, in_maps = _prep_inputs(**inputs)
    key = (NLk, NHk)
    if key not in _cache:
        _cache[key] = build(NLk, NHk)
    nc = _cache[key]
    res = run_bass_kernel_spmd(nc, in_maps, list(range(NCORES)))
    return np.concatenate([res.results[c]["out"] for c in range(NCORES)], 0)
